# revision 1
# baseline (speedup 1.0000x reference)
"""Trainium2 Bass kernel for nn_ATT_learner (retrieval_knn).

Computes rows/cols/values of the top-31 cosine-similarity graph over
N=16384 learned embeddings (relu(f*w0)*w1, L2-normalized), matching
  sim = embn @ embn.T;  vals, inds = top_k(sim, 31);  values = relu(vals)

Device (8 NeuronCores, SPMD, rows sharded 2048/core):
  - embeddings normalized on device from hfeat = relu(f*w0)*w1 (host
    prepares this tiny elementwise input transform + the layout tiles)
  - PE transposes build embnT tiles (float32r); the 2048x16384x512
    similarity matmul runs on the tensor engine in fp32r (full rate)
  - DVE windowed tensor_reduce(max, w=4) + max8 + max_index extract the
    top-8 windows per 1024-column chunk per row
Host: exact fp32 values for the selected candidate columns, final
top-31 by (-value, index) with jax tie semantics, and exact recompute
of the few rows whose candidate set could be incomplete.
"""
import numpy as np

N_NODES = 16384
D = 512
K_OUT = 31
CHUNK = 1024
POOL_W = 4
N_CORES = 8
RPC = N_NODES // N_CORES

_CACHE = {}


def _concourse():
    try:
        import concourse  # noqa: F401
    except ImportError:
        import sys
        for p in ("/opt/trn_rl_repo", "/root/.axon_site/_ro/trn_rl_repo"):
            sys.path.insert(0, p)
    import concourse.mybir as mybir
    import concourse.tile as tile
    from concourse import bacc
    from concourse.masks import make_identity
    from concourse.bass_utils import run_bass_kernel_spmd
    return mybir, tile, bacc, make_identity, run_bass_kernel_spmd


def _build_nc():
    mybir, tile, bacc, make_identity, _ = _concourse()
    F32 = mybir.dt.float32
    F32R = mybir.dt.float32r
    U32 = mybir.dt.uint32
    AF = mybir.ActivationFunctionType

    n_nodes, rows_per_core, d, chunk, pool_w = N_NODES, RPC, D, CHUNK, POOL_W
    dc = d // 128
    n_strips = rows_per_core // 128
    n_ch = n_nodes // chunk
    sub_per_chunk = chunk // 128

    nc = bacc.Bacc(name="knn_kernel")
    feat_ext = nc.declare_dram_parameter("hfeat", [n_nodes, d], F32, isOutput=False)
    own_ext = nc.declare_dram_parameter("own_hfeat", [rows_per_core, d], F32, isOutput=False)
    val_ext = nc.declare_dram_parameter("cand_val", [n_strips, 128, n_ch * 8], F32, isOutput=True)
    idx_ext = nc.declare_dram_parameter("cand_idx", [n_strips, 128, n_ch * 8], U32, isOutput=True)

    feat_ap = feat_ext.ap()
    own_ap = own_ext.ap()

    with tile.TileContext(nc) as tc:
        from contextlib import ExitStack
        with ExitStack() as ctx:
            const_pool = ctx.enter_context(tc.tile_pool(name="const", bufs=1))
            feat_pool = ctx.enter_context(tc.tile_pool(name="feat", bufs=10))
            scratch_pool = ctx.enter_context(tc.tile_pool(name="scratch", bufs=10))
            small_pool = ctx.enter_context(tc.tile_pool(name="small", bufs=3))
            en_pool = ctx.enter_context(tc.tile_pool(name="en", bufs=10))
            rhs_pool = ctx.enter_context(tc.tile_pool(name="rhs", bufs=2))
            lhsT_pool = ctx.enter_context(tc.tile_pool(name="lhsT", bufs=1))
            cand_pool = ctx.enter_context(tc.tile_pool(name="cand", bufs=1))
            psum_tr = ctx.enter_context(tc.tile_pool(name="ptr", bufs=2, space="PSUM"))
            psum_mm = ctx.enter_context(tc.tile_pool(name="pmm", bufs=3, space="PSUM"))

            ident_f32 = const_pool.tile([128, 128], F32, tag="ident_f32", name="ident_f32")
            make_identity(nc, ident_f32[:])
            ident = const_pool.tile([128, 128], F32R, tag="ident", name="ident")
            nc.vector.tensor_copy(ident[:], ident_f32[:])

            def prep_group(src_aps):
                nb = len(src_aps)
                hs = []
                ss = small_pool.tile([128, nb], F32, tag="ss", name="ss")
                for t, src_ap in enumerate(src_aps):
                    h = feat_pool.tile([128, d], F32, tag="ft", name="ft")
                    nc.sync.dma_start(out=h[:], in_=src_ap)
                    sq = scratch_pool.tile([128, d], F32, tag="sq", name="sq")
                    nc.scalar.activation(sq[:], h[:], AF.Square,
                                         accum_out=ss[:, t:t + 1])
                    hs.append(h)
                sr = small_pool.tile([128, nb], F32, tag="sr", name="sr")
                nc.scalar.activation(sr[:], ss[:], AF.Sqrt)
                rn = small_pool.tile([128, nb], F32, tag="rn", name="rn")
                nc.vector.reciprocal(rn[:], sr[:])
                ens = []
                for t, h in enumerate(hs):
                    en = en_pool.tile([128, d], F32R, tag="en", name="en")
                    nc.scalar.activation(en[:], h[:], AF.Copy, scale=rn[:, t:t + 1])
                    ens.append(en)
                return ens

            def transpose_into(en, dst_tiles, col_off):
                for c in range(dc):
                    pst = psum_tr.tile([128, 128], F32R, tag="pst", name="pst")
                    nc.tensor.transpose(pst[:], en[:, c * 128:(c + 1) * 128], ident[:])
                    nc.scalar.activation(
                        dst_tiles[c][:, col_off:col_off + 128], pst[:], AF.Copy
                    )

            lhsT = [lhsT_pool.tile([128, rows_per_core], F32R,
                                   tag=f"lhsT{c}", name=f"lhsT{c}")
                    for c in range(dc)]
            for t0 in range(0, rows_per_core // 128, 8):
                nb = min(8, rows_per_core // 128 - t0)
                ens = prep_group([own_ap[(t0 + t) * 128:(t0 + t + 1) * 128, :]
                                  for t in range(nb)])
                for t, en in enumerate(ens):
                    transpose_into(en, lhsT, (t0 + t) * 128)

            cand_val = [cand_pool.tile([128, n_ch * 8], F32, tag=f"cv{i}", name=f"cv{i}")
                        for i in range(n_strips)]
            cand_idx = [cand_pool.tile([128, n_ch * 8], U32, tag=f"ci{i}", name=f"ci{i}")
                        for i in range(n_strips)]

            def chunk_srcs(j):
                return [feat_ap[j * chunk + t * 128:j * chunk + (t + 1) * 128, :]
                        for t in range(sub_per_chunk)]

            def make_rhs(ens):
                r = [rhs_pool.tile([128, chunk], F32R, tag=f"rhs{c}", name=f"rhs{c}")
                     for c in range(dc)]
                for t, en in enumerate(ens):
                    transpose_into(en, r, t * 128)
                return r

            rhs_cur = make_rhs(prep_group(chunk_srcs(0)))
            sub_n = min(chunk, 512)
            for j in range(n_ch):
                rhs = rhs_cur
                if j + 1 < n_ch:
                    rhs_cur = make_rhs(prep_group(chunk_srcs(j + 1)))
                for s in range(n_strips):
                    ps = psum_mm.tile([128, chunk], F32, tag="ps", name="ps")
                    for half in range(chunk // sub_n):
                        for c in range(dc):
                            nc.tensor.matmul(
                                ps[:, half * sub_n:(half + 1) * sub_n],
                                lhsT[c][:, s * 128:(s + 1) * 128],
                                rhs[c][:, half * sub_n:(half + 1) * sub_n],
                                start=(c == 0),
                                stop=(c == dc - 1),
                            )
                    if pool_w == 1:
                        scan_src = ps[:]
                    else:
                        pooled = scratch_pool.tile([128, chunk // pool_w], F32,
                                                   tag="pooled", name="pooled")
                        nc.vector.tensor_reduce(
                            pooled[:],
                            ps[:].rearrange("p (o w) -> p o w", w=pool_w),
                            axis=mybir.AxisListType.X,
                            op=mybir.AluOpType.max)
                        scan_src = pooled[:]
                    nc.vector.max(out=cand_val[s][:, j * 8:(j + 1) * 8], in_=scan_src)
                    nc.vector.max_index(
                        out=cand_idx[s][:, j * 8:(j + 1) * 8],
                        in_max=cand_val[s][:, j * 8:(j + 1) * 8],
                        in_values=scan_src,
                    )

            for s in range(n_strips):
                nc.sync.dma_start(out=val_ext.ap()[s], in_=cand_val[s][:])
                nc.sync.dma_start(out=idx_ext.ap()[s], in_=cand_idx[s][:])

    nc.finalize()
    return nc


def _run_device(features, w0, w1):
    *_, run_bass_kernel_spmd = _concourse()
    if "nc" not in _CACHE:
        _CACHE["nc"] = _build_nc()
    nc = _CACHE["nc"]
    hfeat = (np.maximum(features * w0, 0) * w1).astype(np.float32)
    in_maps = []
    for c in range(N_CORES):
        in_maps.append({
            "hfeat": hfeat,
            "own_hfeat": np.ascontiguousarray(hfeat[c * RPC:(c + 1) * RPC]),
        })
    last_err = None
    for _attempt in range(3):
        try:
            res = run_bass_kernel_spmd(nc, in_maps, list(range(N_CORES)))
            return res.results
        except Exception as e:  # rare transient NRT device errors
            last_err = e
    raise last_err


def _host_postprocess(cand_val_list, cand_idx_list, features, w0, w1,
                      k_out=K_OUT, chunk=CHUNK, pool_w=POOL_W,
                      flag_margin=1.5e-4, keep_windows=40):
    n, d = features.shape
    h = np.maximum(features * w0, 0) * w1
    norm = np.sqrt((h * h).sum(1, keepdims=True))
    embn = h / np.maximum(norm, 1e-12)

    idxs = np.concatenate([a.reshape(-1, a.shape[-1]) for a in cand_idx_list], 0)
    dval = np.concatenate([a.reshape(-1, a.shape[-1]) for a in cand_val_list], 0)
    n_slots = idxs.shape[1]
    n_ch = n_slots // 8
    assert idxs.shape[0] == n

    base = (np.arange(n_ch, dtype=np.int64) * chunk).repeat(8)
    win_col = idxs.astype(np.int64) * pool_w + base[None, :]

    kw = min(keep_windows, n_slots)
    if kw < n_slots:
        sel = np.argpartition(-dval, kw - 1, axis=1)[:, :kw]
        sel_col = np.take_along_axis(win_col, sel, axis=1)
        sel_dval = np.take_along_axis(dval, sel, axis=1)
        cutoff = sel_dval.min(axis=1)
    else:
        sel = np.broadcast_to(np.arange(n_slots)[None], (n, n_slots)).copy()
        sel_col = win_col
        cutoff = np.full(n, -np.inf, np.float32)

    gidx = (sel_col[:, :, None] + np.arange(pool_w)[None, None, :]).reshape(n, -1)
    ncand = gidx.shape[1]

    exact = np.empty((n, ncand), np.float32)
    B = max(1, (1 << 27) // (ncand * d))
    for b0 in range(0, n, B):
        g = embn[gidx[b0:b0 + B].reshape(-1)].reshape(-1, ncand, d)
        exact[b0:b0 + B] = np.einsum("rcd,rd->rc", g, embn[b0:b0 + B], optimize=True)

    order = np.lexsort((gidx, -exact.astype(np.float64)), axis=1)
    srt_val = np.take_along_axis(exact, order, axis=1)
    srt_idx = np.take_along_axis(gidx, order, axis=1)
    top_val = srt_val[:, :k_out].copy()
    top_idx = srt_idx[:, :k_out].copy()
    v31 = top_val[:, k_out - 1]

    thresh = v31 - flag_margin
    flag = cutoff >= thresh
    wmax_kept = exact.reshape(n, -1, pool_w).max(2)
    near_count = np.zeros((n, n_ch), np.int32)
    sel_chunk = sel // 8
    np.add.at(near_count,
              (np.repeat(np.arange(n), kw).reshape(n, kw), sel_chunk),
              (wmax_kept >= thresh[:, None]).astype(np.int32))
    if kw < n_slots:
        drop_near = (dval >= thresh[:, None])
        kept_mask = np.zeros((n, n_slots), bool)
        np.put_along_axis(kept_mask, sel, True, axis=1)
        drop_near &= ~kept_mask
        near_count += drop_near.reshape(n, n_ch, 8).sum(2)
    flag |= (near_count >= 8).any(1)
    qs = np.sort(idxs.reshape(n, n_ch, 8), axis=2)
    flag |= (np.diff(qs, axis=2) == 0).any(axis=(1, 2))
    flag |= (np.diff(np.sort(srt_idx[:, :k_out + 1], axis=1), axis=1) == 0).any(axis=1)

    nflag = int(flag.sum())
    if nflag:
        frows = np.nonzero(flag)[0]
        sim = embn @ embn[frows].T
        for k, i in enumerate(frows):
            row = sim[:, k]
            o = np.lexsort((np.arange(n), -row.astype(np.float64)))[:k_out]
            top_idx[i] = o
            top_val[i] = row[o]

    rows = np.repeat(np.arange(n, dtype=np.int32), k_out)
    cols = top_idx.reshape(-1).astype(np.int32)
    values = np.maximum(top_val.reshape(-1), 0).astype(np.float32)
    return rows, cols, values


def kernel(features, w0, w1):
    features = np.asarray(features, dtype=np.float32)
    w0 = np.asarray(w0, dtype=np.float32)
    w1 = np.asarray(w1, dtype=np.float32)
    results = _run_device(features, w0, w1)
    cv = [results[c]["cand_val"] for c in range(N_CORES)]
    ci = [results[c]["cand_idx"] for c in range(N_CORES)]
    return _host_postprocess(cv, ci, features, w0, w1)
